# revision 12
# baseline (speedup 1.0000x reference)
"""Multi-head causal attention (B=256,T=256,E=384,H=6,D=64) on 8 trn2 cores.

Sharding: data-parallel over batch (32 batches per core), no collectives.

Per-core pipeline (per batch):
  x [256,384] --PE transpose--> xT [384,256]
  QT = wq_all.T @ xT   [384,256]  (heads stacked on partitions, 2 heads/chunk)
  KT = wk_all.T @ xT   [384,256]
  V  = xT.T @ wv_all   [256,384]  (natural layout)
  per head h:
    S[q,k] = QT_h.T @ KT_h  (K=64 contraction), causal mask added into PSUM
             via an extra identity @ maskneg matmul (fp16, -60000 additive)
    stage = exp((S+mask)/8) on ACT (fp16 out), accum_out -> row sums
    rec = 1/sums (DVE, [128,2])
    PT[k,q] = stage.T @ diag(rec)  -- transpose & normalize in one fp16 matmul
    OT_h[d,q] = V_h.T-style: lhsT=V[:,h*64:+64], rhs=PT  (f32r)
  out[t,:] = OT_all.T @ w_proj + b  (f32r) -> DMA out

All fp32 matmuls use the float32r PE mode (1 cycle/row at N>=256).
"""

import numpy as np
from contextlib import ExitStack

import concourse.bass as bass
from concourse import bacc
import concourse.mybir as mybir
import concourse.tile as tile
from concourse.masks import make_identity

F32 = mybir.dt.float32
F32R = mybir.dt.float32r
F16 = mybir.dt.float16

B, T, E, H, D = 256, 256, 384, 6, 64
N_CORES = 8
NB = B // N_CORES  # batches per core
EC = E // 128      # 3 e-chunks
MC = (H * D) // 128  # 3 head-dim chunks (2 heads each)

Exp = mybir.ActivationFunctionType.Exp


def r(ap):
    return ap.bitcast(F32R)


def build(nb=NB):
    nc = bacc.Bacc("TRN2", debug=False, num_devices=N_CORES)
    x = nc.dram_tensor("x", [nb, T, E], F32, kind="ExternalInput").ap()
    wq = nc.dram_tensor("wq", [H, E, D], F32, kind="ExternalInput").ap()
    wk = nc.dram_tensor("wk", [H, E, D], F32, kind="ExternalInput").ap()
    wv = nc.dram_tensor("wv", [H, E, D], F32, kind="ExternalInput").ap()
    wp = nc.dram_tensor("w_proj", [H * D, E], F32, kind="ExternalInput").ap()
    bp = nc.dram_tensor("b_proj", [E], F32, kind="ExternalInput").ap()
    out = nc.dram_tensor("out", [nb, T, E], F32, kind="ExternalOutput").ap()

    with tile.TileContext(nc) as tc, ExitStack() as ctx:
        const = ctx.enter_context(tc.tile_pool(name="const", bufs=1))

        # --- constants ---
        ident = const.tile([128, 128], F32, tag="ident")
        make_identity(nc, ident[:])
        ident16 = const.tile([128, 128], F16, tag="ident16")
        make_identity(nc, ident16[:])
        # additive causal mask for a diagonal 128x128 block:
        # valid (q>=k i.e. p>=f): 0, else -60000
        maskneg = const.tile([128, 128], F16, tag="maskneg")
        nc.gpsimd.memset(maskneg[:], 0.0)
        nc.gpsimd.affine_select(
            out=maskneg[:],
            in_=maskneg[:],
            compare_op=mybir.AluOpType.is_ge,
            fill=-60000.0,
            base=0,
            pattern=[[-1, 128]],
            channel_multiplier=1,
        )
        bias_bc = const.tile([128, E], F32, tag="bias")
        nc.sync.dma_start(bias_bc[:], bp.unsqueeze(0).broadcast_to([128, E]))

        # --- weights: w{q,k,v}_all[e, h*64+d] chunked on e; wp chunked on f ---
        wq_sb, wk_sb, wv_sb, wp_sb = [], [], [], []
        for ec in range(EC):
            for (dst, src, tg) in ((wq_sb, wq, "wq"), (wk_sb, wk, "wk"),
                                   (wv_sb, wv, "wv")):
                t_ = const.tile([128, H * D], F32R, tag=f"{tg}{ec}")
                nc.gpsimd.dma_start(
                    t_[:].rearrange("p (h d) -> p h d", h=H),
                    src.transpose([1, 0, 2])[ec * 128:(ec + 1) * 128, :, :])
                dst.append(t_)
            t_ = const.tile([128, E], F32R, tag=f"wp{ec}")
            nc.gpsimd.dma_start(t_[:], wp[ec * 128:(ec + 1) * 128, :])
            wp_sb.append(t_)

        # --- pools ---
        xnp = ctx.enter_context(tc.tile_pool(name="xn", bufs=4))
        xtp = ctx.enter_context(tc.tile_pool(name="xt", bufs=6))
        qkp = ctx.enter_context(tc.tile_pool(name="qk", bufs=12))
        vp = ctx.enter_context(tc.tile_pool(name="v", bufs=4))
        stp = ctx.enter_context(tc.tile_pool(name="st", bufs=8))
        smp = ctx.enter_context(tc.tile_pool(name="sm", bufs=8))
        dgp = ctx.enter_context(tc.tile_pool(name="dg", bufs=8))
        ptp = ctx.enter_context(tc.tile_pool(name="pt", bufs=8))
        otp = ctx.enter_context(tc.tile_pool(name="ot", bufs=6))
        obp = ctx.enter_context(tc.tile_pool(name="ob", bufs=4))

        psA = ctx.enter_context(tc.tile_pool(name="psA", bufs=3, space="PSUM"))
        psB = ctx.enter_context(tc.tile_pool(name="psB", bufs=3, space="PSUM"))
        psO = ctx.enter_context(tc.tile_pool(name="psO", bufs=2, space="PSUM"))

        for b in range(nb):
            # ---- load x, build xT ----
            xn = []
            for tcc in range(2):
                t_ = xnp.tile([128, E], F32, tag="xn")
                nc.sync.dma_start(t_[:], x[b, tcc * 128:(tcc + 1) * 128, :])
                xn.append(t_)
            xt = []
            for ec in range(EC):
                t_ps = psB.tile([128, 256], F32, tag="psB")
                nc.tensor.transpose(
                    t_ps[:, 0:128], xn[0][:, ec * 128:(ec + 1) * 128], ident[:])
                nc.tensor.transpose(
                    t_ps[:, 128:256], xn[1][:, ec * 128:(ec + 1) * 128], ident[:])
                t_ = xtp.tile([128, 256], F32R, tag="xt")
                nc.vector.tensor_copy(t_[:], t_ps[:])
                xt.append(t_)

            # ---- QT / KT ----
            QT, KT = [], []
            for (w_sb, dst) in ((wq_sb, QT), (wk_sb, KT)):
                for mc in range(MC):
                    q_ps = psA.tile([128, 256], F32, tag="psA")
                    for ec in range(EC):
                        nc.tensor.matmul(
                            q_ps[:],
                            w_sb[ec][:, mc * 128:(mc + 1) * 128],
                            xt[ec][:],
                            start=(ec == 0), stop=(ec == EC - 1))
                    t_ = qkp.tile([128, 256], F32R, tag="qk")
                    nc.vector.tensor_copy(t_[:], q_ps[:])
                    dst.append(t_)

            # ---- V (natural [t, h*64+d]) ----
            V = []
            for tcc in range(2):
                v_ps = psB.tile([128, H * D], F32, tag="psB")
                for ec in range(EC):
                    nc.tensor.matmul(
                        v_ps[:],
                        xt[ec][:, tcc * 128:(tcc + 1) * 128],
                        wv_sb[ec][:],
                        start=(ec == 0), stop=(ec == EC - 1))
                t_ = vp.tile([128, H * D], F16, tag="v")
                nc.vector.tensor_copy(t_[:], v_ps[:])
                V.append(t_)

            # ---- heads ----
            OT = [None] * MC
            ot_ps = None
            for h in range(H):
                mc, half = divmod(h, 2)
                p0 = half * 64
                qt = QT[mc]
                kt = KT[mc]
                # S: one PSUM bank per q-chunk (HW: only one matmul
                # accumulation group may live in a bank at a time)
                s_qc = []
                for qc in range(2):
                    s_ps = psA.tile([128, 256], F32, tag="psA")
                    nc.tensor.matmul(
                        s_ps[:],
                        qt[p0:p0 + 64, qc * 128:(qc + 1) * 128],
                        kt[p0:p0 + 64, :],
                        start=True, stop=False, skip_group_check=True)
                    # additive causal mask on the diagonal block
                    nc.tensor.matmul(
                        s_ps[:, qc * 128:qc * 128 + 128],
                        ident16[:], maskneg[:],
                        start=False, stop=True, skip_group_check=True)
                    s_qc.append(s_ps)

                sums = smp.tile([128, 2], F32, tag="sums")
                stage0 = stp.tile([128, 256], F16, tag="stage")
                nc.scalar.activation(stage0[:, 0:128], s_qc[0][:, 0:128], Exp,
                                     scale=0.125, accum_out=sums[:, 0:1])
                stage1 = stp.tile([128, 256], F16, tag="stage")
                nc.scalar.activation(stage1[:], s_qc[1][:], Exp,
                                     scale=0.125, accum_out=sums[:, 1:2])

                rec = smp.tile([128, 2], F32, tag="rec")
                nc.vector.reciprocal(rec[:], sums[:])
                dg0 = dgp.tile([128, 128], F16, tag="diag")
                nc.vector.tensor_scalar_mul(dg0[:], ident16[:], rec[:, 0:1])
                dg1 = dgp.tile([128, 128], F16, tag="diag")
                nc.vector.tensor_scalar_mul(dg1[:], ident16[:], rec[:, 1:2])

                # PT = stage.T @ diag(rec): [k, q] normalized
                ptk0_ps = psB.tile([128, 256], F32, tag="psB")
                nc.tensor.matmul(ptk0_ps[:, 0:128], stage0[:, 0:128], dg0[:],
                                 start=True, stop=True, skip_group_check=True)
                nc.tensor.matmul(ptk0_ps[:, 128:256], stage1[:, 0:128], dg1[:],
                                 start=True, stop=True, skip_group_check=True)
                ptk1_ps = psB.tile([128, 128], F32, tag="psB")
                nc.tensor.matmul(ptk1_ps[:], stage1[:, 128:256], dg1[:],
                                 start=True, stop=True)

                ptk0 = ptp.tile([128, 256], F16, tag="pt")
                nc.vector.tensor_copy(ptk0[:], ptk0_ps[:])
                ptk1 = ptp.tile([128, 256], F16, tag="pt")
                nc.vector.tensor_copy(
                    ptk1[:, 0:128], nc.const_aps.tensor(0.0, (128, 128), F32))
                nc.vector.tensor_copy(ptk1[:, 128:256], ptk1_ps[:])

                # OT pair bank: head-even rows 0:64, head-odd rows 64:128
                if half == 0:
                    ot_ps = psO.tile([128, 256], F32, tag="psO")
                nc.tensor.matmul(ot_ps[p0:p0 + 64, :],
                                 V[0][:, h * 64:(h + 1) * 64], ptk0[:],
                                 start=True, stop=False, skip_group_check=True)
                nc.tensor.matmul(ot_ps[p0:p0 + 64, :],
                                 V[1][:, h * 64:(h + 1) * 64], ptk1[:],
                                 start=False, stop=True, skip_group_check=True)
                if half == 1:
                    t_ = otp.tile([128, 256], F32R, tag="ot")
                    nc.vector.tensor_copy(t_[:], ot_ps[:])
                    OT[mc] = t_

            # ---- output projection ----
            for tcc in range(2):
                pr_ps = psB.tile([128, E], F32, tag="psB")
                for mc in range(MC):
                    nc.tensor.matmul(
                        pr_ps[:],
                        OT[mc][:, tcc * 128:(tcc + 1) * 128],
                        wp_sb[mc][:],
                        start=(mc == 0), stop=(mc == MC - 1))
                ob = obp.tile([128, E], F32, tag="ob")
                nc.vector.tensor_add(ob[:], pr_ps[:], bias_bc[:])
                nc.sync.dma_start(out[b, tcc * 128:(tcc + 1) * 128, :], ob[:])

    nc.compile()
    return nc


_NC_CACHE = {}


def kernel(x, wq, wk, wv, w_proj, b_proj):
    x = np.ascontiguousarray(np.asarray(x, dtype=np.float32))
    wq = np.ascontiguousarray(np.asarray(wq, dtype=np.float32))
    wk = np.ascontiguousarray(np.asarray(wk, dtype=np.float32))
    wv = np.ascontiguousarray(np.asarray(wv, dtype=np.float32))
    w_proj = np.ascontiguousarray(np.asarray(w_proj, dtype=np.float32))
    b_proj = np.ascontiguousarray(np.asarray(b_proj, dtype=np.float32))

    from concourse.bass_utils import run_bass_kernel_spmd

    if NB not in _NC_CACHE:
        _NC_CACHE[NB] = build(NB)
    nc = _NC_CACHE[NB]

    in_maps = []
    for c in range(N_CORES):
        in_maps.append({
            "x": np.ascontiguousarray(x[c * NB:(c + 1) * NB]),
            "wq": wq, "wk": wk, "wv": wv,
            "w_proj": w_proj, "b_proj": b_proj,
        })
    res = run_bass_kernel_spmd(nc, in_maps, core_ids=list(range(N_CORES)))
    return np.concatenate([r_["out"] for r_ in res.results], axis=0)
